# revision 1
# baseline (speedup 1.0000x reference)
"""Trainium2 Bass kernel for nn_Attention_30562987278646.

Sharding: 8 cores = 4 batches x 2 head-groups (4 heads each).
Per core: LN(q/k/v slice) -> project with W_in columns of its heads ->
score matrices (cosine + covariance + margin-variance) -> out = S @ f_v
-> partial @ W_out rows. Host sums the 2 head-group partials per batch.

Exact identities used:
 - LN: ln_g folded into W_in on host (W' = diag(g) W); ln_b @ W_in added
   via K=1 accumulating matmuls on device. Device applies (x - mu) * r only.
 - cov term: qc . kc = dots - d*mq*mk -> rank-1 outer product folded as
   extra contraction rows (K=66 matmul: 64 f-rows + means row + ones row).
 - var term: GAMMA=1 and cosine <= 1 mathematically, so
   relu(1 - cos) == 1 - cos; mean_m(1 - cos_nm) = 1 - colsum(cos_nm)/N,
   and colsum(cos_nm)[n] = (sum_m fk_n[:,m]) . fq_n[:,n] -- one tiny matmul.
 - cos_nm == cosine_sim (norms >> 1e-12), computed once.

Everything runs in d-major (transposed) layout so score matrices come out
transposed (S^T[m,n]) and feed the out-stage matmul directly.
"""

import sys
import numpy as np

for _p in ("/opt/trn_rl_repo", "/root/.axon_site/_ro/trn_rl_repo"):
    if _p not in sys.path:
        sys.path.append(_p)

HEADS = 8
DIM_HEAD = 64
LN_EPS = 1e-5
B, N, DIM = 4, 1024, 512
HG = 2                      # head groups (shards along heads)
HPG = HEADS // HG           # heads per group = 4
IG = HPG * DIM_HEAD         # inner dim per group = 256
NT = N // 128               # 8 n-tiles
NC = N // 512               # 2 n-chunks
CC = DIM // 128             # 4 c-chunks


def _build_nc(cos_w: float, cov_w: float, var_w: float):
    import concourse.bass as bass
    import concourse.bacc as bacc
    import concourse.tile as tile
    from concourse import mybir

    f32 = mybir.dt.float32
    f32r = mybir.dt.float32r
    AF = mybir.ActivationFunctionType
    AX = mybir.AxisListType

    def r(ap):
        return ap.bitcast(f32r)

    nc = bacc.Bacc(target_bir_lowering=False, debug=False)
    _lp = nc.allow_low_precision(reason="f32r is 4-byte storage, not low precision")
    _lp.__enter__()

    xin_d = {
        "xq": nc.declare_dram_parameter("xq", [N, DIM], f32, isOutput=False),
        "xk": nc.declare_dram_parameter("xk", [N, DIM], f32, isOutput=False),
        "xv": nc.declare_dram_parameter("xv", [N, DIM], f32, isOutput=False),
    }
    wf = nc.declare_dram_parameter("wf", [DIM, IG], f32, isOutput=False)
    bw = nc.declare_dram_parameter("bw", [64, IG], f32, isOutput=False)
    wo = nc.declare_dram_parameter("wo", [IG, DIM], f32, isOutput=False)
    ident = nc.declare_dram_parameter("ident", [128, 128], f32, isOutput=False)
    sel = nc.declare_dram_parameter("sel", [128, 2], f32, isOutput=False)
    e1 = nc.declare_dram_parameter("e1", [64, 512], f32, isOutput=False)
    eb = nc.declare_dram_parameter("eb", [128, 128], f32, isOutput=False)
    out = nc.declare_dram_parameter("out", [N, DIM], f32, isOutput=True)

    with tile.TileContext(nc) as tc, \
         tc.tile_pool(name="persist", bufs=1) as P, \
         tc.tile_pool(name="stt", bufs=4) as STP, \
         tc.tile_pool(name="small", bufs=6) as SM, \
         tc.tile_pool(name="osb", bufs=8) as OSB, \
         tc.tile_pool(name="psu", bufs=4, space="PSUM") as PSU, \
         tc.tile_pool(name="psc", bufs=2, space="PSUM") as PSC, \
         tc.tile_pool(name="pst", bufs=2, space="PSUM") as PT:

        # ---- constants / weights in SBUF ----
        id_stage = P.tile([128, 128], f32, tag="id_stage", name="id_stage")
        nc.gpsimd.dma_start(out=id_stage, in_=ident[:, :])
        id_sb = P.tile([128, 128], f32, tag="id", name="id_sb")
        nc.scalar.activation(id_sb, id_stage, AF.Copy)
        sel_sb = P.tile([128, 2], f32r, tag="sel", name="sel_sb")
        nc.gpsimd.dma_start(out=sel_sb, in_=sel[:, :].bitcast(f32r))
        e1_sb = P.tile([64, 512], f32r, tag="e1", name="e1_sb")
        nc.gpsimd.dma_start(out=e1_sb, in_=e1[:, :].bitcast(f32r))
        eb_sb = P.tile([128, 128], f32r, tag="eb", name="eb_sb")
        nc.gpsimd.dma_start(out=eb_sb, in_=eb[:, :].bitcast(f32r))
        bw_sb = P.tile([64, IG], f32r, tag="bw", name="bw_sb")
        nc.gpsimd.dma_start(out=bw_sb, in_=bw[:, :].bitcast(f32r))
        eps_sb = P.tile([128, 1], f32, tag="eps", name="eps_sb")
        nc.vector.memset(eps_sb, LN_EPS)
        vw_sb = P.tile([1, 1], f32, tag="vw", name="vw_sb")
        nc.vector.memset(vw_sb, var_w)
        wf_sb = [P.tile([128, IG], f32r, tag=f"wf{c}", name=f"wf{c}") for c in range(CC)]
        for c in range(CC):
            nc.gpsimd.dma_start(out=wf_sb[c], in_=wf[c * 128:(c + 1) * 128, :].bitcast(f32r))
        wo_sb = [P.tile([64, DIM], f32r, tag=f"wo{j}", name=f"wo{j}") for j in range(4)]
        for j in range(4):
            nc.gpsimd.dma_start(out=wo_sb[j], in_=wo[j * 64:(j + 1) * 64, :].bitcast(f32r))

        # ---- persistent activations (projection outputs) ----
        fTq = [P.tile([128, N], f32r, tag=f"fTq{hp}", name=f"fTq{hp}") for hp in range(2)]
        fTk = [P.tile([128, N], f32r, tag=f"fTk{hp}", name=f"fTk{hp}") for hp in range(2)]
        fv_sb = [P.tile([128, IG], f32r, tag=f"fv{mt}", name=f"fv{mt}") for mt in range(NT)]

        # ======== stages A+B under a scoped pool for the xT tiles ========
        with tc.tile_pool(name="xtp", bufs=1) as XT, \
             tc.tile_pool(name="xin", bufs=4) as XIN, \
             tc.tile_pool(name="xdma", bufs=24) as XD:
            xT = {t: [XT.tile([128, N], f32r, tag=f"xT{t}{c}", name=f"xT{t}{c}")
                      for c in range(CC)] for t in ("xq", "xk", "xv")}

            # stage A: load, LN, transpose to c-major
            for t in ("xq", "xk", "xv"):
                for nt in range(NT):
                    xt = XD.tile([128, DIM], f32, tag="xt")
                    nc.gpsimd.dma_start(
                        out=xt, in_=xin_d[t][nt * 128:(nt + 1) * 128, :])
                    stats = SM.tile([128, nc.vector.BN_STATS_DIM], f32,
                                    tag="bns")
                    nc.vector.bn_stats(out=stats, in_=xt)
                    mv = SM.tile([128, nc.vector.BN_AGGR_DIM], f32, tag="bna")
                    nc.vector.bn_aggr(out=mv, in_=stats)
                    std = SM.tile([128, 1], f32, tag="std")
                    nc.scalar.activation(std, mv[:, 1:2], AF.Sqrt, bias=eps_sb)
                    rin = SM.tile([128, 1], f32, tag="rin")
                    nc.vector.reciprocal(rin, std)
                    nmr = SM.tile([128, 1], f32, tag="nmr")
                    nc.vector.tensor_mul(nmr, mv[:, 0:1], rin)
                    nc.vector.tensor_scalar_mul(nmr, nmr, -1.0)
                    zt = XIN.tile([128, DIM], f32, tag="zt")
                    nc.vector.tensor_scalar_mul(zt, xt, rin)
                    xln = XIN.tile([128, DIM], f32, tag="xln")
                    nc.scalar.activation(xln, zt, AF.Identity, bias=nmr)
                    for c in range(CC):
                        pt = PT.tile([128, 128], f32, tag="pt")
                        nc.tensor.transpose(
                            pt, xln[:, c * 128:(c + 1) * 128], id_sb)
                        nc.scalar.activation(
                            xT[t][c][:, nt * 128:(nt + 1) * 128], pt,
                            AF.Copy)

            # stage B: projections (fp32r)
            for tname, fT in (("xq", fTq), ("xk", fTk)):
                for hp in range(2):
                    for ncx in range(NC):
                        pf = PSU.tile([128, 512], f32, tag="big")
                        for c in range(CC):
                            nc.tensor.matmul(
                                pf,
                                r(wf_sb[c][:, hp * 128:(hp + 1) * 128]),
                                r(xT[tname][c][:, ncx * 512:(ncx + 1) * 512]),
                                start=(c == 0), stop=False)
                        nc.tensor.matmul(
                            pf, r(bw_sb[:, hp * 128:(hp + 1) * 128]),
                            r(e1_sb[0:64, 0:512]), start=False, stop=True)
                        nc.vector.tensor_copy(
                            fT[hp][:, ncx * 512:(ncx + 1) * 512], pf)
            for mt in range(NT):
                pf = PSU.tile([128, IG], f32, tag="big")
                for c in range(CC):
                    nc.tensor.matmul(
                        pf, r(xT["xv"][c][:, mt * 128:(mt + 1) * 128]),
                        r(wf_sb[c]), start=(c == 0), stop=False)
                nc.tensor.matmul(pf, r(e1_sb[0:64, 0:128]), r(bw_sb),
                                 start=False, stop=True)
                nc.vector.tensor_copy(fv_sb[mt], pf)

        # ---- stages C-E under a second persist pool (xT memory now free) ----
        with tc.tile_pool(name="p2", bufs=1) as P2:
            fqn = [P2.tile([128, N], f32r, tag=f"fqn{hp}", name=f"fqn{hp}")
                   for hp in range(2)]
            fkn = [P2.tile([128, N], f32r, tag=f"fkn{hp}", name=f"fkn{hp}")
                   for hp in range(2)]
            fqc = [P2.tile([128, N], f32r, tag=f"fqc{hp}", name=f"fqc{hp}")
                   for hp in range(2)]
            # per-head [1,N] stat rows packed at 32-aligned partition bases.
            # Matmul pairs need EQUAL bases on both operands, so each quantity
            # gets its own tile with heads 0-2 at rows 0/32/64, head 3 at row 0
            # of a sibling tile. ONESP provides an all-ones row at each base.
            RP = [P2.tile([97, N], f32r, tag=f"RP{q}", name=f"RP{q}")
                  for q in range(3)]
            RPB = [P2.tile([33, N], f32r, tag=f"RPB{q}", name=f"RPB{q}")
                   for q in range(3)]
            ONESP = P2.tile([97, 128], f32r, tag="ONESP", name="ONESP")
            zst = P2.tile([128, N], f32, tag="zst", name="zst")
            nc.vector.memset(zst, 0.0)
            for q in range(3):
                nc.scalar.activation(RP[q], zst[0:97, :], AF.Copy)
                nc.scalar.activation(RPB[q], zst[0:33, :], AF.Copy)
            ost = P2.tile([97, 128], f32, tag="ost", name="ost")
            nc.vector.memset(ost, 0.0)
            for b in (0, 32, 64):
                nc.vector.memset(ost[b:b + 1, :], 1.0)
            nc.scalar.activation(ONESP, ost, AF.Copy)

            def row(q, h):
                if h < 3:
                    return RP[q][32 * h:32 * h + 1, :]
                return RPB[q][0:1, :]

            def blk(q, h):
                if h < 3:
                    return RP[q][32 * h:32 * h + 32, :]
                return RPB[q][0:32, :]

            def ones_blk(h):
                if h < 3:
                    return ONESP[32 * h:32 * h + 32, 0:128]
                return e1_sb[0:32, 0:128]

            MK, NMQ, VR = 0, 1, 2
            fks = [P2.tile([128, 1], f32r, tag=f"fks{hp}", name=f"fks{hp}")
                   for hp in range(2)]
            oTh = [P2.tile([64, N], f32r, tag=f"oTh{h}", name=f"oTh{h}")
                   for h in range(HPG)]

            # ======== stage C: stats, norms ========
            with tc.tile_pool(name="rows", bufs=1) as RW:
                qsr = [RW.tile([128, N], f32r, tag=f"qsr{hp}", name=f"qsr{hp}")
                       for hp in range(2)]
                ksr = [RW.tile([128, N], f32r, tag=f"ksr{hp}", name=f"ksr{hp}")
                       for hp in range(2)]

                for t_ in qsr + ksr:
                    nc.scalar.activation(t_, zst, AF.Copy)

                def srow(tiles, h):
                    return tiles[h // 2][(h % 2) * 64:(h % 2) * 64 + 1, :]
                # per-head column sums of f and f^2 via M=1 selector matmuls
                for fT, dsq, dsm in ((fTq, qsr, NMQ), (fTk, ksr, MK)):
                    for hp in range(2):
                        sq = STP.tile([128, N], f32r, tag="sq")
                        nc.scalar.activation(sq, fT[hp], AF.Square)
                        for hj in range(2):
                            h = 2 * hp + hj
                            for ncx in range(NC):
                                cs = slice(ncx * 512, (ncx + 1) * 512)
                                p1 = PSU.tile([1, 512], f32, tag="big")
                                nc.tensor.matmul(p1, r(sel_sb[:, hj:hj + 1]),
                                                 r(fT[hp][:, cs]),
                                                 start=True, stop=True)
                                nc.vector.tensor_copy(row(dsm, h)[:, cs], p1)
                                p2 = PSU.tile([1, 512], f32, tag="big")
                                nc.tensor.matmul(p2, r(sel_sb[:, hj:hj + 1]),
                                                 r(sq[:, cs]),
                                                 start=True, stop=True)
                                nc.vector.tensor_copy(srow(dsq, h)[:, cs], p2)
                for h in range(HPG):
                    # qsr: sum(q^2)->cos_w/qn ; ksr: sum(k^2)->1/kn (in place)
                    qr, kr = srow(qsr, h), srow(ksr, h)
                    nc.scalar.activation(qr, qr, AF.Sqrt)
                    nc.vector.reciprocal(qr, qr)
                    nc.vector.tensor_scalar_mul(qr, qr, cos_w)
                    nc.scalar.activation(kr, kr, AF.Sqrt)
                    nc.vector.reciprocal(kr, kr)
                    nc.vector.tensor_scalar_mul(row(MK, h), row(MK, h),
                                                1.0 / DIM_HEAD)
                    nc.vector.tensor_scalar_mul(row(NMQ, h), row(NMQ, h),
                                                -cov_w / DIM_HEAD)
                # broadcast per-head rows across 64 partitions -> fqn/fkn
                for hp in range(2):
                    for ncx in range(NC):
                        cs = slice(ncx * 512, (ncx + 1) * 512)
                        pb = PSU.tile([128, 512], f32, tag="big")
                        nc.tensor.matmul(pb, r(eb_sb),
                                         r(qsr[hp][:, cs]),
                                         start=True, stop=True)
                        nc.vector.tensor_mul(fqn[hp][:, cs],
                                             fTq[hp][:, cs], pb)
                        pb2 = PSU.tile([128, 512], f32, tag="big")
                        nc.tensor.matmul(pb2, r(eb_sb),
                                         r(ksr[hp][:, cs]),
                                         start=True, stop=True)
                        nc.vector.tensor_mul(fkn[hp][:, cs],
                                             fTk[hp][:, cs], pb2)
                    nc.vector.tensor_scalar_mul(fqc[hp], fTq[hp],
                                                cov_w / DIM_HEAD)
                    nc.vector.reduce_sum(fks[hp], fkn[hp], axis=AX.X)
            # var rows: vr = var_w * (1 - colsum(cos)/N)
            for h in range(HPG):
                hp, ds = h // 2, (h % 2) * 64
                for ncx in range(NC):
                    cs = slice(ncx * 512, (ncx + 1) * 512)
                    pv = PSU.tile([1, 512], f32, tag="big")
                    nc.tensor.matmul(
                        pv, r(fks[hp][ds:ds + 64, 0:1]),
                        r(fqn[hp][ds:ds + 64, cs]),
                        start=True, stop=True)
                    nc.scalar.activation(
                        row(VR, h)[:, cs], pv, AF.Identity,
                        bias=vw_sb, scale=-(var_w / (N * cos_w)))

            # ======== stage D: scores + out-stage ========
            di = 0
            for ncx in range(NC):
                cs = slice(ncx * 512, (ncx + 1) * 512)
                for hp in range(2):
                    for hj in range(2):
                        h = 2 * hp + hj
                        ds = (h % 2) * 64
                        po = PSU.tile([64, 512], f32, tag="big")
                        for mt in range(NT):
                            ms = slice(mt * 128, (mt + 1) * 128)
                            pss = PSC.tile([128, 512], f32, tag="pss")
                            nc.tensor.matmul(
                                pss, r(fkn[hp][ds:ds + 64, ms]),
                                r(fqn[hp][ds:ds + 64, cs]),
                                start=True, stop=False)
                            nc.tensor.matmul(
                                pss, r(fTk[hp][ds:ds + 64, ms]),
                                r(fqc[hp][ds:ds + 64, cs]),
                                start=False, stop=False)
                            nc.tensor.matmul(
                                pss, r(blk(MK, h)[:, ms]),
                                r(blk(NMQ, h)[:, cs]),
                                start=False, stop=False)
                            nc.tensor.matmul(
                                pss, r(ones_blk(h)),
                                r(blk(VR, h)[:, cs]),
                                start=False, stop=True)
                            st = STP.tile([128, 512], f32r, tag="st")
                            if di % 2 == 0:
                                nc.vector.tensor_copy(st, pss)
                            else:
                                nc.scalar.activation(st, pss, AF.Copy)
                            di += 1
                            nc.tensor.matmul(
                                po,
                                r(fv_sb[mt][:, h * 64:(h + 1) * 64]),
                                r(st), start=(mt == 0), stop=(mt == NT - 1))
                        nc.scalar.activation(
                            oTh[h][:, ncx * 512:(ncx + 1) * 512], po, AF.Copy)

            # ======== stage E: W_out projection + store ========
            for nt in range(NT):
                pf = PSU.tile([128, 512], f32, tag="big")
                for j in range(4):
                    nc.tensor.matmul(
                        pf, r(oTh[j][:, nt * 128:(nt + 1) * 128]),
                        r(wo_sb[j]), start=(j == 0), stop=(j == 3))
                ob = OSB.tile([128, 512], f32, tag="ob")
                nc.vector.tensor_copy(ob, pf)
                nc.gpsimd.dma_start(out=out[nt * 128:(nt + 1) * 128, :],
                                    in_=ob)

    _lp.__exit__(None, None, None)
    nc.compile()
    return nc


def _prep(q, k, v, ln_g, ln_b, W_in, W_out, b_out, cov_w_raw, var_w_raw):
    q = np.asarray(q, np.float32)
    k = np.asarray(k, np.float32)
    v = np.asarray(v, np.float32)
    ln_g = np.asarray(ln_g, np.float32)
    ln_b = np.asarray(ln_b, np.float32)
    W_in = np.asarray(W_in, np.float32)
    W_out = np.asarray(W_out, np.float32)

    cov_w = float(1.0 / (1.0 + np.exp(-np.float64(cov_w_raw))))
    var_w = float(1.0 / (1.0 + np.exp(-np.float64(var_w_raw))))
    cos_w = 1.0 - cov_w - var_w

    nc = _build_nc(cos_w, cov_w, var_w)

    W_f = (ln_g[:, None] * W_in).astype(np.float32)      # [512, 512]
    bW = (ln_b @ W_in).astype(np.float32)                # [512]
    ident = np.eye(128, dtype=np.float32)
    sel = np.zeros((128, 2), np.float32)
    sel[:64, 0] = 1.0
    sel[64:, 1] = 1.0
    e1 = np.zeros((64, 512), np.float32)
    e1[0, :] = 1.0
    eb = np.zeros((128, 128), np.float32)
    eb[0, :64] = 1.0
    eb[64, 64:] = 1.0

    in_maps = []
    for core in range(8):
        b, g = core // HG, core % HG
        in_maps.append({
            "xq": np.ascontiguousarray(q[b]),
            "xk": np.ascontiguousarray(k[b]),
            "xv": np.ascontiguousarray(v[b]),
            "wf": np.ascontiguousarray(W_f[:, g * IG:(g + 1) * IG]),
            "bw": np.ascontiguousarray(
                np.concatenate([bW[None, g * IG:(g + 1) * IG],
                                np.zeros((63, IG), np.float32)], axis=0)),
            "wo": np.ascontiguousarray(W_out[g * IG:(g + 1) * IG, :]),
            "ident": ident, "sel": sel, "e1": e1, "eb": eb,
        })
    return nc, in_maps


def kernel(q, k, v, ln_g, ln_b, W_in, W_out, b_out, cov_w_raw, var_w_raw):
    from concourse.bass_utils import run_bass_kernel_spmd

    b_out = np.asarray(b_out, np.float32)
    nc, in_maps = _prep(q, k, v, ln_g, ln_b, W_in, W_out, b_out,
                        cov_w_raw, var_w_raw)
    res = run_bass_kernel_spmd(nc, in_maps, list(range(8)))
    parts = [res.results[c]["out"] for c in range(8)]
    out = np.stack([parts[2 * b] + parts[2 * b + 1] + b_out
                    for b in range(B)])
    return out.astype(np.float32)

